# revision 18
# baseline (speedup 1.0000x reference)
"""AdaptiveESN Trainium2 kernel.

Echo State Network: B=64, T=2048, D=128, H=512, leaky a=0.26.
    h_t = (1-a) h_{t-1} + a tanh(x_t W_in^T + b_in + h_{t-1} W_res^T + b_res)
    y_t = h_t W_ro^T

Strategy: data-parallel over batch across 8 NeuronCores (8 rows each).
Per core the scan is sequential in T; each step is a batched matvec
z = (a*W_res) h~ + W_in x_t with h~ = h/a (scale folded into the weights so
the leaky blend is h~_new = (1-a) h~ + tanh(z + b), one tensor_scalar +
one tensor_tensor on DVE). State kept transposed in SBUF as bf16
[H(4x128 part chunks), 8]; W_res tiles are the stationary matmul operand
(bf16 -> fast weight load). Readout is fused per time-chunk on-device.

Layouts (host-prepped, per core c):
    xt   bf16 [128, T*8]   xt[d, t*8+b] = x[8c+b, t, d]
    wres bf16 [128, 2048]  tile (j,i) at cols (j*4+i)*128: (a W_res).T block
    win  bf16 [128, 512]   W_in.T
    wro  bf16 [128, 512]   tile j at cols j*128: (a W_ro).T block
    bias f32  [128, 4]     (b_in + b_res) chunk i in col i
    out  f32  [128, T*8]   out[d, t*8+b] = y[8c+b, t, d]
"""
import sys

if "/opt/trn_rl_repo" not in sys.path:
    sys.path.insert(0, "/opt/trn_rl_repo")

import numpy as np
import ml_dtypes

import concourse.bass as bass
from concourse import bacc
import concourse.mybir as mybir
import concourse.tile as tile
from concourse.bass_utils import run_bass_kernel_spmd

try:
    import jax

    jax.config.update("jax_compilation_cache_dir", "/tmp/jax_neff_cache")
    jax.config.update("jax_persistent_cache_min_compile_time_secs", 10)
except Exception:
    pass

B, T, D, H = 64, 2048, 128, 512
LEAKY = 0.26
NCORES = 8
BL = B // NCORES          # batch rows per core
NCH = H // 128            # H chunks (partition tiles)
TC = 128                  # time steps per states chunk
BF16 = mybir.dt.bfloat16
F32 = mybir.dt.float32

TRACE = False             # test harness can flip this for profiling
WRES_FP8 = False          # fp8e4m3 W_res (2x faster weight loads, ~1e-2 extra err)
_last_results = None      # BassKernelResults of the most recent run


def build(t_total=T, tc=TC, reps=1, probe=None, wres_fp8=False, ps7=False):
    """Build the per-core Bacc graph (same graph on all 8 cores).

    reps > 1 wraps the whole scan in a hardware For_i loop that re-runs it
    (same instruction count) — used to measure pure on-device time via
    wall-clock deltas between two reps values.

    probe: timing-only structural variants (WRONG math, never for output):
      "noldw"  - only 2 of 4 W_res K-chunks per round (12 pairs vs 20)
      "noldw16" - drop only j=3 (16 pairs vs 20)
      "noh23"  - j=2,3 rhs reads the zero tile (same burst, shorter chain)
      "zrhs"   - ALL matmul rhs read the zero tile (no cross-engine deps into PE)
      "nepi"   - zrhs AND no ACT/TT/y at all (pure PE burst rate, 20 pairs)
      "nepi12" - nepi with only 12 pairs
      "nott"   - ACT writes states directly, no blend TT / y op
      "noro"   - skip the readout matmuls/DMAs
    """
    nchunks = t_total // tc
    assert nchunks * tc == t_total
    assert nchunks % 2 == 0 or nchunks == 1 or reps == 1
    W = NCH * BL              # step-major state width: one step's [h chunks x batch]

    nc = bacc.Bacc(None, target_bir_lowering=False)
    xt_e = nc.declare_dram_parameter("xt", [128, t_total * BL], BF16, isOutput=False)
    wres_dt = mybir.dt.float8e4 if wres_fp8 else BF16
    wres_e = nc.declare_dram_parameter("wres", [128, 16 * 128], wres_dt, isOutput=False)
    win_e = nc.declare_dram_parameter("win", [128, NCH * 128], BF16, isOutput=False)
    wro_e = nc.declare_dram_parameter("wro", [128, NCH * 128], BF16, isOutput=False)
    bias_e = nc.declare_dram_parameter("bias", [128, NCH], F32, isOutput=False)
    out_e = nc.declare_dram_parameter("out", [128, t_total * BL], F32, isOutput=True)

    with tile.TileContext(nc) as tc_ctx:
        with (
            tc_ctx.tile_pool(name="const", bufs=1) as const_pool,
            tc_ctx.tile_pool(name="p", bufs=(12 if ps7 else 8)) as p_pool,
            tc_ctx.tile_pool(name="y", bufs=8) as y_pool,
            tc_ctx.tile_pool(name="ostage", bufs=3) as o_pool,
            tc_ctx.tile_pool(name="scan_ps", bufs=(7 if ps7 else 6), space=bass.MemorySpace.PSUM) as ps_pool,
            tc_ctx.tile_pool(name="ro_ps", bufs=(1 if ps7 else 2), space=bass.MemorySpace.PSUM) as ro_pool,
        ):
            xt_sb = const_pool.tile([128, t_total * BL], BF16)
            wres_sb = const_pool.tile([128, 16 * 128], wres_dt)
            win_sb = const_pool.tile([128, NCH * 128], BF16)
            wro_sb = const_pool.tile([128, NCH * 128], BF16)
            bias_sb = const_pool.tile([128, NCH], F32)
            h0_sb = const_pool.tile([128, W], BF16)
            # states, step-major: column s*W + i*BL + b  (i = H chunk, b = batch)
            st = [
                const_pool.tile([128, tc * W], BF16, name=f"st{k}", tag=f"st{k}")
                for k in range(2)
            ]

            nc.sync.dma_start(wres_sb[:], wres_e[:])
            nc.sync.dma_start(win_sb[:], win_e[:])
            nc.sync.dma_start(wro_sb[:], wro_e[:])
            nc.sync.dma_start(bias_sb[:], bias_e[:])
            nc.sync.dma_start(xt_sb[:], xt_e[:])
            nc.vector.memset(h0_sb[:], 0.0)
            if probe in ("nepi", "nepi12"):
                # no blend writes states in these probes; readout still reads it
                nc.vector.memset(st[0][:], 0.0)
                nc.vector.memset(st[1][:], 0.0)

            def scan_body(_iv=None):
                for c in range(nchunks):
                    cur, prv = c % 2, (c - 1) % 2
                    for s in range(tc):
                        t = c * tc + s
                        if t == 0:
                            hprev = h0_sb[:]
                        elif s == 0:
                            hprev = st[prv][:, (tc - 1) * W : tc * W]
                        else:
                            hprev = st[cur][:, (s - 1) * W : s * W]

                        zero_rhs = probe in ("zrhs", "nepi", "nepi12")
                        no_epi = probe in ("nepi", "nepi12")
                        npairs = {"noldw": 3, "nepi12": 3, "noldw16": 4}.get(probe, 5)

                        def hcol(j):
                            if zero_rhs or (probe == "noh23" and j >= 2):
                                return h0_sb[:, j * BL : (j + 1) * BL]
                            return hprev[:, j * BL : (j + 1) * BL]

                        xcol = xt_sb[:, t * BL : (t + 1) * BL]
                        if probe != "nott" and not no_epi:
                            # y = (1-a) * h_{t-1}, all chunks in one DVE op (off-path)
                            y_t = y_pool.tile([128, W], BF16)
                            nc.vector.tensor_scalar_mul(y_t[:], hprev, 1.0 - LEAKY)
                        for i in range(NCH):
                            ps = ps_pool.tile([128, BL], F32)
                            # j-order (0,1,2,win,3): defer the h[3] consumption
                            ops = [
                                (wres_sb[:, (0 * NCH + i) * 128 : (0 * NCH + i + 1) * 128], hcol(0)),
                                (wres_sb[:, (1 * NCH + i) * 128 : (1 * NCH + i + 1) * 128], hcol(1)),
                                (wres_sb[:, (2 * NCH + i) * 128 : (2 * NCH + i + 1) * 128], hcol(2)),
                                (win_sb[:, i * 128 : (i + 1) * 128], xcol),
                                (wres_sb[:, (3 * NCH + i) * 128 : (3 * NCH + i + 1) * 128], hcol(3)),
                            ]
                            if npairs == 3:
                                ops = ops[:2] + [ops[3]]
                            elif npairs == 4:
                                ops = ops[:4]
                            for k, (lhsT, rhs) in enumerate(ops):
                                nc.tensor.matmul(
                                    ps[:], lhsT, rhs,
                                    start=(k == 0), stop=(k == len(ops) - 1))
                            if no_epi:
                                continue
                            st_col = st[cur][:, s * W + i * BL : s * W + (i + 1) * BL]
                            if probe == "nott":
                                nc.scalar.activation(
                                    st_col, ps[:], mybir.ActivationFunctionType.Tanh,
                                    bias=bias_sb[:, i : i + 1],
                                )
                            else:
                                p_t = p_pool.tile([128, BL], BF16)
                                nc.scalar.activation(
                                    p_t[:], ps[:], mybir.ActivationFunctionType.Tanh,
                                    bias=bias_sb[:, i : i + 1],
                                )
                                nc.vector.tensor_tensor(
                                    st_col,
                                    y_t[:, i * BL : (i + 1) * BL], p_t[:],
                                    op=mybir.AluOpType.add,
                                )
                    if probe == "noro":
                        continue
                    # fused readout of chunk c: out = (a W_ro).T @ states
                    # states chunk j for steps [s0, s0+ns): strided AP over st
                    base = c * tc * BL
                    st_v = st[cur].rearrange("p (s w) -> p s w", w=W)
                    ns = 512 // BL  # steps per readout tile
                    for n in range(0, tc, ns):
                        nw = min(ns, tc - n)
                        rps = ro_pool.tile([128, 512], F32)
                        for j in range(NCH):
                            nc.tensor.matmul(
                                rps[:, : nw * BL],
                                wro_sb[:, j * 128 : (j + 1) * 128],
                                st_v[:, n : n + nw, j * BL : (j + 1) * BL],
                                start=(j == 0),
                                stop=(j == NCH - 1),
                            )
                        ostage = o_pool.tile([128, 512], F32)
                        nc.vector.tensor_copy(ostage[:, : nw * BL], rps[:, : nw * BL])
                        nc.sync.dma_start(
                            out_e[:, base + n * BL : base + (n + nw) * BL],
                            ostage[:, : nw * BL],
                        )

            if reps == 1:
                scan_body()
            else:
                with tc_ctx.For_i(0, reps, 1) as _i:
                    scan_body(_i)

    nc.compile()
    return nc


def host_prep(x, W_in, b_in, W_res, b_res, W_ro, t_total=T, wres_fp8=False):
    """Produce the per-core in_maps (host-side layout/dtype prep only)."""
    a = np.float32(LEAKY)
    AT = (a * W_res).T.astype(np.float32)                     # [in, out]
    wres_np_dt = ml_dtypes.float8_e4m3 if wres_fp8 else ml_dtypes.bfloat16
    wres = (
        AT.reshape(NCH, 128, NCH, 128).transpose(1, 0, 2, 3).reshape(128, 16 * 128)
    ).astype(wres_np_dt)
    win = W_in.T.astype(ml_dtypes.bfloat16)                   # [128, 512]
    R = (a * W_ro).T.astype(np.float32)                       # [512, 128]
    wro = R.reshape(NCH, 128, 128).transpose(1, 0, 2).reshape(128, NCH * 128).astype(
        ml_dtypes.bfloat16
    )
    bias = (b_in + b_res).astype(np.float32).reshape(NCH, 128).T.copy()  # [128, 4]

    in_maps = []
    for c in range(NCORES):
        xl = x[c * BL : (c + 1) * BL, :t_total, :]            # [8, t, 128]
        xt = np.ascontiguousarray(xl.transpose(2, 1, 0).reshape(128, t_total * BL))
        in_maps.append({
            "xt": xt.astype(ml_dtypes.bfloat16),
            "wres": wres, "win": win, "wro": wro, "bias": bias,
        })
    return in_maps


_nc_cache = {}


def kernel(x, W_in, b_in, W_res, b_res, W_ro):
    """Full inputs in, full output out ([B, T, D] float32)."""
    global _last_results
    x, W_in, b_in, W_res, b_res, W_ro = (
        np.asarray(t, dtype=np.float32) for t in (x, W_in, b_in, W_res, b_res, W_ro)
    )
    t_total = x.shape[1]
    if t_total not in _nc_cache:
        _nc_cache[t_total] = build(t_total=t_total, tc=min(TC, t_total), wres_fp8=WRES_FP8)
    nc = _nc_cache[t_total]

    in_maps = host_prep(x, W_in, b_in, W_res, b_res, W_ro, t_total=t_total, wres_fp8=WRES_FP8)
    res = run_bass_kernel_spmd(nc, in_maps, list(range(NCORES)), trace=TRACE)
    _last_results = res

    out = np.empty((B, t_total, D), dtype=np.float32)
    for c in range(NCORES):
        oc = res.results[c]["out"]                            # [128, t*8]
        out[c * BL : (c + 1) * BL] = oc.reshape(128, t_total, BL).transpose(2, 1, 0)
    return out
